# revision 14
# baseline (speedup 1.0000x reference)
"""Trainium2 Bass kernel for nn_CCMetrics (connected-component soft-Dice).

Math
----
Reference per sample: probs = softmax(y_pred, ch axis 1) with C=2 channels,
one-hot labels y in {0,1}.  Per-voxel channel sums collapse:
  psum_v = tsum_v = 1          (softmax / one-hot sum to 1 over channels)
  inter_v = probs[true_ch] = sigmoid((2y-1) * (z1 - z0))
So per segment id k (voronoi component, 0..64):
  inter_k = sum of sigmoid values v over voxels with id k
  cnt_k   = voxel count with id k
  dice_k  = (2*inter_k + eps) / (2*cnt_k + eps)
  score   = mean over present k in 1..64;  output = mean over batch.

Device algorithm (per core, data-parallel: 4M voxels / 8 cores)
---------------------------------------------------------------
Build y = g + 0.5 + v (fp16; g = component id).  Two reduction families,
one scalar per bin k = 1..64:
  T_k  = #{g >= k}                      -> cnt_k  = T_k - T_{k+1}
  R_k  = sum relu(y - (k+0.5))
       = sum_{j>=k} (j-k)*cnt_j + V_k,  V_k = sum_{g>=k} v
       -> inter_k = V_k - V_{k+1}
Bins are spread over three parallel paths:
  * PE path: DVE tensor_scalar fp16 4x produces the mask/relu tile
    (~1.4us), TensorE folds it into PSUM row r via a one-hot-column
    stationary E_r (8 matmuls of FD=512, ~2.1us, overlapped).
    One [128,512] PSUM bank accumulates every PE bin; a single DVE fold
    tree at the end yields all PE-bin scalars at once.
  * ACT path: activation with fused accumulate (sigmoid staircase on g
    for T bins, relu on y for R bins), ~3.9us per bin.
  * DVE solo path (T bins only): is_ge + partial fold tree to a [P,256]
    remnant per bin, one shared global cascade (baseline scheme).
Host combines per-core partials in float64 and runs the exact recovery.
"""

import os
import sys

import numpy as np

for _p in ("/opt/trn_rl_repo",):
    if os.path.isdir(_p) and _p not in sys.path:
        sys.path.insert(0, _p)

import concourse.bass as bass  # noqa: E402
from concourse import bacc, mybir, tile  # noqa: E402
from concourse import bass_utils  # noqa: E402

NUM_COMP = 64
EPS = 1e-5
B, C, H, W, D = 2, 2, 128, 128, 128
N = H * W * D
NCORES = 8
CORES_PER_SAMPLE = NCORES // B
CHUNK = N // CORES_PER_SAMPLE
P = 128
F = CHUNK // P
KMAX = NUM_COMP

# ---- bin assignment over the three paths -----------------------------------
N_SOLO = int(os.environ.get("CC_NSOLO", "14"))      # T bins on DVE solo path
N_ACT_T = int(os.environ.get("CC_NACT_T", "19"))    # T bins on ACT
N_ACT_R = int(os.environ.get("CC_NACT_R", "19"))    # R bins on ACT
NMASK = int(os.environ.get("CC_NMASK", "6"))        # mask rotation depth

def _spread(pool_ks, n):
    """pick n spread-out values from pool_ks"""
    if n <= 0:
        return []
    idx = np.linspace(0, len(pool_ks) - 1, n).round().astype(int)
    return [pool_ks[i] for i in sorted(set(idx.tolist()))]

_allk = list(range(1, KMAX + 1))
SOLO_T = _spread(_allk, N_SOLO)
_rem_t = [k for k in _allk if k not in SOLO_T]
ACT_T = _spread(_rem_t, N_ACT_T)
PE_T = [k for k in _rem_t if k not in ACT_T]
ACT_R = _spread(_allk, N_ACT_R)
PE_R = [k for k in _allk if k not in ACT_R]
N_PE = len(PE_T) + len(PE_R)

TRACE = False
_prog_cache = {}


def _build_program():
    nc = bacc.Bacc(
        "TRN2",
        target_bir_lowering=False,
        debug=False,
        enable_asserts=False,
        num_devices=NCORES,
    )
    f32 = mybir.dt.float32
    f16 = mybir.dt.float16

    g_d = nc.dram_tensor("g", [P, F], f16, kind="ExternalInput").ap()
    # z0 | z1 | y01 packed into one tensor -> one DMA trigger
    zzy_d = nc.dram_tensor("zzy", [P, 3 * F], f16, kind="ExternalInput").ap()
    NOUT = 1 + len(ACT_T) + len(ACT_R) + len(SOLO_T)
    out_d = nc.dram_tensor("out", [P, NOUT], f32, kind="ExternalOutput").ap()

    Alu = mybir.AluOpType
    Act = mybir.ActivationFunctionType

    with tile.TileContext(nc) as tc:
        with (
            tc.tile_pool(name="main", bufs=1) as pool,
            tc.tile_pool(name="psum", bufs=1, space=bass.MemorySpace.PSUM) as pp,
        ):
            g = pool.tile([P, F], f16)
            zzy = pool.tile([P, 3 * F], f16)
            # g first: the early T bins only need g (+ gpsimd-built biases)
            nc.sync.dma_start(out=g[:], in_=g_d[:])
            nc.sync.dma_start(out=zzy[:], in_=zzy_d[:])
            z0 = zzy[:, 0:F]
            z1 = zzy[:, F:2 * F]
            y01 = zzy[:, 2 * F:3 * F]
            # per-bin ACT bias columns, built by the otherwise-idle GPSIMD
            biast = pool.tile([P, max(len(ACT_T), 1)], f32)
            for j, k in enumerate(ACT_T):
                nc.gpsimd.memset(biast[:, j:j + 1], -30.0 * (k - 0.5))
            biasr = pool.tile([P, max(len(ACT_R), 1)], f32)
            for j, k in enumerate(ACT_R):
                nc.gpsimd.memset(biasr[:, j:j + 1], -(k + 0.5))

            # E_r stationaries ([128,128] f16, column r ones) via idle GPSIMD
            Es = []
            for r in range(N_PE):
                Er = pool.tile([P, P], f16, name=f"E{r}")
                nc.gpsimd.memset(Er[:], 0.0)
                nc.gpsimd.memset(Er[:, r:r + 1], 1.0)
                Es.append(Er)

            masks = [pool.tile([P, F], f16, name=f"mk{i}") for i in range(NMASK)]
            ps = pp.tile([P, 512], f32)
            acc_act = pool.tile([P, max(len(ACT_T) + len(ACT_R), 1)], f32)
            trash = pool.tile([P, F], f16)

            n_mm_total = N_PE * 8
            mm = {"n": 0, "mi": 0}

            def emit_pe_bin(kind, k):
                m = masks[mm["mi"] % NMASK]
                mm["mi"] += 1
                if kind == "T":
                    nc.vector.tensor_scalar(
                        out=m[:], in0=g[:], scalar1=float(k) - 0.5,
                        scalar2=None, op0=Alu.is_ge)
                else:
                    nc.vector.tensor_scalar(
                        out=m[:], in0=y[:], scalar1=float(k) + 0.5,
                        scalar2=0.0, op0=Alu.subtract, op1=Alu.max)
                r = mm["n"] // 8
                for c in range(8):
                    nc.tensor.matmul(
                        ps[:, :],
                        Es[r][:, :],
                        m[:, c * 512:(c + 1) * 512],
                        start=(mm["n"] == 0),
                        stop=(mm["n"] == n_mm_total - 1),
                    )
                    mm["n"] += 1

            act_ci = {"n": 0}

            def emit_act_t(j):
                nc.scalar.activation(
                    out=trash[:], in_=g[:], func=Act.Sigmoid,
                    bias=biast[:, j:j + 1], scale=30.0,
                    accum_out=acc_act[:, act_ci["n"]:act_ci["n"] + 1])
                act_ci["n"] += 1

            def emit_act_r(j):
                nc.scalar.activation(
                    out=trash2[:], in_=y[:], func=Act.Relu,
                    bias=biasr[:, j:j + 1], scale=1.0,
                    accum_out=acc_act[:, act_ci["n"]:act_ci["n"] + 1])
                act_ci["n"] += 1

            # solo-path tiles (baseline partial-tree + remnant scheme)
            nsolo = len(SOLO_T)
            cmp16 = pool.tile([P, F], f16)
            fb1 = pool.tile([P, F // 2], f16)
            fb2 = pool.tile([P, F // 4], f16)
            fb3 = pool.tile([P, F // 8], f16)
            RW = F // 16
            remn = pool.tile([P, max(nsolo, 1) * RW], f16)

            def emit_solo_bin(bi, k):
                nc.vector.tensor_scalar(
                    out=cmp16[:], in0=g[:], scalar1=float(k) - 0.5,
                    scalar2=None, op0=Alu.is_ge)
                nc.vector.tensor_add(fb1[:], cmp16[:, :F // 2], cmp16[:, F // 2:])
                nc.vector.tensor_add(fb2[:], fb1[:, :F // 4], fb1[:, F // 4:])
                nc.vector.tensor_add(fb3[:], fb2[:, :F // 8], fb2[:, F // 8:])
                nc.vector.tensor_add(
                    remn[:, bi * RW:(bi + 1) * RW],
                    fb3[:, :F // 16], fb3[:, F // 16:])

            # ---------------- schedule ----------------
            # ACT: a few T bins fill time until t' is ready, then sigmoid(v),
            # then the rest.
            n_act_t_early = min(4, len(ACT_T))
            for j in range(n_act_t_early):
                emit_act_t(j)

            # DVE: early PE T-bins while z0/z1/y01 arrive (interleave solo)
            pe_seq = [("T", k) for k in PE_T] + [("R", k) for k in PE_R]
            solo_seq = list(enumerate(SOLO_T))
            pe_i = 0
            solo_i = 0
            n_early = 5
            for _ in range(min(n_early, len(pe_seq))):
                emit_pe_bin(*pe_seq[pe_i]); pe_i += 1

            # preprocessing: d = z1 - z0 ; t' = (y01 - 0.5) * d ; v = sigmoid(2 t')
            # (scalar_tensor_tensor runs 1x on DVE; tensor_scalar 4x +
            # tensor_tensor 2x pairs are faster)
            d = pool.tile([P, F], f16)
            nc.vector.tensor_sub(d[:], z1, z0)
            ysh = pool.tile([P, F], f16)
            nc.vector.tensor_scalar(
                out=ysh[:], in0=y01, scalar1=-0.5, scalar2=None, op0=Alu.add)
            tp = pool.tile([P, F], f16)
            nc.vector.tensor_mul(tp[:], ysh[:], d[:])
            v = pool.tile([P, F], f16)
            nc.scalar.activation(
                out=v[:], in_=tp[:], func=Act.Sigmoid, scale=2.0)
            # y = (g + 0.5) + v
            gsh = pool.tile([P, F], f16)
            nc.vector.tensor_scalar(
                out=gsh[:], in0=g[:], scalar1=0.5, scalar2=None, op0=Alu.add)
            y = pool.tile([P, F], f16)
            nc.vector.tensor_add(y[:], gsh[:], v[:])
            trash2 = pool.tile([P, F], f16)

            # rest of ACT bins
            for j in range(n_act_t_early, len(ACT_T)):
                emit_act_t(j)
            for j in range(len(ACT_R)):
                emit_act_r(j)

            # rest of DVE: interleave PE-bin gens with solo bins
            while pe_i < len(pe_seq) or solo_i < len(solo_seq):
                for _ in range(6):
                    if pe_i < len(pe_seq):
                        emit_pe_bin(*pe_seq[pe_i]); pe_i += 1
                if solo_i < len(solo_seq):
                    emit_solo_bin(*solo_seq[solo_i]); solo_i += 1

            # global cascade over solo remnants: [P, nsolo, w] fold to [P, nsolo]
            gb = remn
            w = RW
            while w > 1:
                half = w // 2
                src = gb[:].rearrange("p (g d) -> p g d", d=w)
                dt_lvl = f16 if half >= 2 else f32
                dst_t = pool.tile([P, max(nsolo, 1) * half], dt_lvl,
                                  name=f"gfold{w}")
                dst = dst_t[:].rearrange("p (g d) -> p g d", d=half)
                nc.vector.tensor_add(dst, src[:, :, :half], src[:, :, half:])
                gb = dst_t
                w = half

            # PSUM fold: copy to SBUF, tree 512 -> 1 (all PE bins at once)
            sb512 = pool.tile([P, 512], f32)
            nc.vector.tensor_copy(sb512[:], ps[:])
            cur = sb512
            wv = 512
            while wv > 1:
                nxt = pool.tile([P, wv // 2], f32, name=f"pf{wv//2}")
                nc.vector.tensor_add(nxt[:], cur[:, 0:wv // 2], cur[:, wv // 2:wv])
                cur = nxt
                wv //= 2

            # stage all outputs into one contiguous tile -> single DMA
            na = len(ACT_T) + len(ACT_R)
            stage = pool.tile([P, NOUT], f32)
            nc.vector.tensor_copy(stage[:, 0:1], cur[:])
            nc.vector.tensor_copy(stage[:, 1:1 + na], acc_act[:, 0:na])
            nc.vector.tensor_copy(stage[:, 1 + na:1 + na + nsolo], gb[:])
            nc.sync.dma_start(out=out_d[:], in_=stage[:])

    nc.compile()
    return nc


def _get_program():
    key = ("prog", tuple(SOLO_T), tuple(ACT_T), tuple(ACT_R))
    if key not in _prog_cache:
        _prog_cache[key] = _build_program()
    return _prog_cache[key]


def kernel(y_pred: np.ndarray, y: np.ndarray, voronoi: np.ndarray) -> np.ndarray:
    f16 = np.float16

    y_pred = np.asarray(y_pred, dtype=np.float32)
    y = np.asarray(y)
    voronoi = np.asarray(voronoi)

    nc = _get_program()

    in_maps = []
    for c in range(NCORES):
        b = c // CORES_PER_SAMPLE
        q = c % CORES_PER_SAMPLE
        sl = slice(q * CHUNK, (q + 1) * CHUNK)
        zp = y_pred[b].reshape(C, N)
        zzy = np.empty((P, 3 * F), dtype=f16)
        zzy[:, 0:F] = zp[0, sl].astype(f16).reshape(P, F)
        zzy[:, F:2 * F] = zp[1, sl].astype(f16).reshape(P, F)
        zzy[:, 2 * F:3 * F] = y[b, 0].reshape(N)[sl].astype(f16).reshape(P, F)
        in_maps.append({
            "g": np.ascontiguousarray(
                voronoi[b].reshape(N)[sl]).astype(f16).reshape(P, F),
            "zzy": zzy,
        })

    res = bass_utils.run_bass_kernel_spmd(
        nc, in_maps, core_ids=list(range(NCORES)), trace=TRACE,
    )
    kernel.last_results = res

    # ---- host-side gather/unshard: combine per-core partials (f64) ----
    T = np.zeros((B, KMAX + 2), dtype=np.float64)
    R = np.zeros((B, KMAX + 2), dtype=np.float64)
    pe_seq = [("T", k) for k in PE_T] + [("R", k) for k in PE_R]
    na = len(ACT_T) + len(ACT_R)
    for c in range(NCORES):
        b = c // CORES_PER_SAMPLE
        out = np.asarray(res.results[c]["out"], dtype=np.float64)
        # PE bins: column 0, row r = bin scalar
        for r, (kind, k) in enumerate(pe_seq):
            val = out[r, 0]
            if kind == "T":
                T[b, k] += val
            else:
                R[b, k] += val
        # ACT bins: per-partition sums in cols 1..na
        acts = out[:, 1:1 + na].sum(axis=0)
        for j, k in enumerate(ACT_T):
            T[b, k] += acts[j]
        for j, k in enumerate(ACT_R):
            R[b, k] += acts[len(ACT_T) + j]
        # solo bins
        solo = out[:, 1 + na:1 + na + len(SOLO_T)].sum(axis=0)
        for j, k in enumerate(SOLO_T):
            T[b, k] += solo[j]

    scores = []
    ks = np.arange(1, KMAX + 1)
    for b in range(B):
        Tb = T[b].copy()
        Tb[KMAX + 1] = 0.0
        cnt = np.round(Tb[1:KMAX + 1] - Tb[2:KMAX + 2])  # cnt_k, k=1..64
        # V_k = R_k - sum_{j>=k} (j-k)*cnt_j  for k=1..65 (V_65 = 0)
        V = np.zeros(KMAX + 2)
        for k in range(1, KMAX + 1):
            j = np.arange(k, KMAX + 1)
            V[k] = R[b, k] - np.sum((j - k) * cnt[j - 1])
        inter = V[1:KMAX + 1] - V[2:KMAX + 2]
        dice = (2.0 * inter + EPS) / (2.0 * cnt + EPS)
        present = cnt > 0
        n_present = max(present.sum(), 1)
        scores.append(np.where(present, dice, 0.0).sum() / n_present)

    return np.float32(np.mean(scores))


# revision 18
# speedup vs baseline: 1.0581x; 1.0581x over previous
"""Trainium2 Bass kernel for nn_CCMetrics (connected-component soft-Dice).

Math
----
Reference per sample: probs = softmax(y_pred, ch axis 1) with C=2 channels,
one-hot labels y in {0,1}.  Per-voxel channel sums collapse:
  psum_v = tsum_v = 1          (softmax / one-hot sum to 1 over channels)
  inter_v = probs[true_ch] = sigmoid((2y-1) * (z1 - z0))
So per segment id k (voronoi component, 0..64):
  inter_k = sum of sigmoid values v over voxels with id k
  cnt_k   = voxel count with id k
  dice_k  = (2*inter_k + eps) / (2*cnt_k + eps)
  score   = mean over present k in 1..64;  output = mean over batch.

Device algorithm (per core, data-parallel: 4M voxels / 8 cores)
---------------------------------------------------------------
Build y = g + 0.5 + v (fp16; g = component id).  Two reduction families,
one scalar per bin k = 1..64:
  T_k  = #{g >= k}                      -> cnt_k  = T_k - T_{k+1}
  R_k  = sum relu(y - (k+0.5))
       = sum_{j>=k} (j-k)*cnt_j + V_k,  V_k = sum_{g>=k} v
       -> inter_k = V_k - V_{k+1}
Bins are spread over three parallel paths:
  * PE path: DVE tensor_scalar fp16 4x produces the mask/relu tile
    (~1.4us), TensorE folds it into PSUM row r via a one-hot-column
    stationary E_r (8 matmuls of FD=512, ~2.1us, overlapped).
    One [128,512] PSUM bank accumulates every PE bin; a single DVE fold
    tree at the end yields all PE-bin scalars at once.
  * ACT path: activation with fused accumulate (sigmoid staircase on g
    for T bins, relu on y for R bins), ~3.9us per bin.
  * DVE solo path (T bins only): is_ge + partial fold tree to a [P,256]
    remnant per bin, one shared global cascade (baseline scheme).
Host combines per-core partials in float64 and runs the exact recovery.
"""

import os
import sys

import numpy as np

for _p in ("/opt/trn_rl_repo",):
    if os.path.isdir(_p) and _p not in sys.path:
        sys.path.insert(0, _p)

import concourse.bass as bass  # noqa: E402
from concourse import bacc, mybir, tile  # noqa: E402
from concourse import bass_utils  # noqa: E402

NUM_COMP = 64
EPS = 1e-5
B, C, H, W, D = 2, 2, 128, 128, 128
N = H * W * D
NCORES = 8
CORES_PER_SAMPLE = NCORES // B
CHUNK = N // CORES_PER_SAMPLE
P = 128
F = CHUNK // P
KMAX = NUM_COMP

# ---- bin assignment over the three paths -----------------------------------
N_SOLO = int(os.environ.get("CC_NSOLO", "14"))      # T bins on DVE solo path
N_ACT_T = int(os.environ.get("CC_NACT_T", "19"))    # T bins on ACT
N_ACT_R = int(os.environ.get("CC_NACT_R", "19"))    # R bins on ACT
NMASK = int(os.environ.get("CC_NMASK", "6"))        # mask rotation depth

def _spread(pool_ks, n):
    """pick n spread-out values from pool_ks"""
    if n <= 0:
        return []
    idx = np.linspace(0, len(pool_ks) - 1, n).round().astype(int)
    return [pool_ks[i] for i in sorted(set(idx.tolist()))]

_allk = list(range(1, KMAX + 1))
SOLO_T = _spread(_allk, N_SOLO)
_rem_t = [k for k in _allk if k not in SOLO_T]
ACT_T = _spread(_rem_t, N_ACT_T)
PE_T = [k for k in _rem_t if k not in ACT_T]
ACT_R = _spread(_allk, N_ACT_R)
PE_R = [k for k in _allk if k not in ACT_R]
N_PE = len(PE_T) + len(PE_R)

TRACE = False
_prog_cache = {}


def _build_program():
    nc = bacc.Bacc(
        "TRN2",
        target_bir_lowering=False,
        debug=False,
        enable_asserts=False,
        num_devices=NCORES,
    )
    f32 = mybir.dt.float32
    f16 = mybir.dt.float16

    g_d = nc.dram_tensor("g", [P, F], f16, kind="ExternalInput").ap()
    z0_d = nc.dram_tensor("z0", [P, F], f16, kind="ExternalInput").ap()
    z1_d = nc.dram_tensor("z1", [P, F], f16, kind="ExternalInput").ap()
    y01_d = nc.dram_tensor("y01", [P, F], f16, kind="ExternalInput").ap()
    biast_d = nc.dram_tensor("biast", [P, max(len(ACT_T), 1)], f32,
                             kind="ExternalInput").ap()
    biasr_d = nc.dram_tensor("biasr", [P, max(len(ACT_R), 1)], f32,
                             kind="ExternalInput").ap()
    NOUT = 1 + len(ACT_T) + len(ACT_R) + len(SOLO_T)
    out_d = nc.dram_tensor("out", [P, NOUT], f32, kind="ExternalOutput").ap()

    Alu = mybir.AluOpType
    Act = mybir.ActivationFunctionType

    with tile.TileContext(nc) as tc:
        with (
            tc.tile_pool(name="main", bufs=1) as pool,
            tc.tile_pool(name="psum", bufs=1, space=bass.MemorySpace.PSUM) as pp,
        ):
            g = pool.tile([P, F], f16)
            z0t = pool.tile([P, F], f16)
            z1t = pool.tile([P, F], f16)
            y01t = pool.tile([P, F], f16)
            biast = pool.tile([P, max(len(ACT_T), 1)], f32)
            biasr = pool.tile([P, max(len(ACT_R), 1)], f32)
            # g first: the early T bins only need g (+ tiny biases)
            nc.sync.dma_start(out=g[:], in_=g_d[:])
            nc.sync.dma_start(out=biast[:], in_=biast_d[:])
            nc.sync.dma_start(out=biasr[:], in_=biasr_d[:])
            nc.sync.dma_start(out=z0t[:], in_=z0_d[:])
            nc.sync.dma_start(out=z1t[:], in_=z1_d[:])
            nc.sync.dma_start(out=y01t[:], in_=y01_d[:])
            z0 = z0t[:]
            z1 = z1t[:]
            y01 = y01t[:]

            # E_r stationaries ([128,128] f16, column r ones) via idle GPSIMD
            Es = []
            for r in range(N_PE):
                Er = pool.tile([P, P], f16, name=f"E{r}")
                nc.gpsimd.memset(Er[:], 0.0)
                nc.gpsimd.memset(Er[:, r:r + 1], 1.0)
                Es.append(Er)

            masks = [pool.tile([P, F], f16, name=f"mk{i}") for i in range(NMASK)]
            ps = pp.tile([P, 512], f32)
            acc_act = pool.tile([P, max(len(ACT_T) + len(ACT_R), 1)], f32)
            trash = pool.tile([P, F], f16)

            n_mm_total = N_PE * 8
            mm = {"n": 0, "mi": 0}

            def emit_pe_bin(kind, k):
                m = masks[mm["mi"] % NMASK]
                mm["mi"] += 1
                if kind == "T":
                    nc.vector.tensor_scalar(
                        out=m[:], in0=g[:], scalar1=float(k) - 0.5,
                        scalar2=None, op0=Alu.is_ge)
                else:
                    nc.vector.tensor_scalar(
                        out=m[:], in0=y[:], scalar1=float(k) + 0.5,
                        scalar2=0.0, op0=Alu.subtract, op1=Alu.max)
                r = mm["n"] // 8
                for c in range(8):
                    nc.tensor.matmul(
                        ps[:, :],
                        Es[r][:, :],
                        m[:, c * 512:(c + 1) * 512],
                        start=(mm["n"] == 0),
                        stop=(mm["n"] == n_mm_total - 1),
                    )
                    mm["n"] += 1

            act_ci = {"n": 0}

            def emit_act_t(j):
                nc.scalar.activation(
                    out=trash[:], in_=g[:], func=Act.Sigmoid,
                    bias=biast[:, j:j + 1], scale=30.0,
                    accum_out=acc_act[:, act_ci["n"]:act_ci["n"] + 1])
                act_ci["n"] += 1

            def emit_act_r(j):
                nc.scalar.activation(
                    out=trash2[:], in_=y[:], func=Act.Relu,
                    bias=biasr[:, j:j + 1], scale=1.0,
                    accum_out=acc_act[:, act_ci["n"]:act_ci["n"] + 1])
                act_ci["n"] += 1

            # solo-path tiles (baseline partial-tree + remnant scheme)
            nsolo = len(SOLO_T)
            cmp16 = pool.tile([P, F], f16)
            fb1 = pool.tile([P, F // 2], f16)
            fb2 = pool.tile([P, F // 4], f16)
            fb3 = pool.tile([P, F // 8], f16)
            RW = F // 16
            remn = pool.tile([P, max(nsolo, 1) * RW], f16)

            def emit_solo_bin(bi, k):
                nc.vector.tensor_scalar(
                    out=cmp16[:], in0=g[:], scalar1=float(k) - 0.5,
                    scalar2=None, op0=Alu.is_ge)
                nc.vector.tensor_add(fb1[:], cmp16[:, :F // 2], cmp16[:, F // 2:])
                nc.vector.tensor_add(fb2[:], fb1[:, :F // 4], fb1[:, F // 4:])
                nc.vector.tensor_add(fb3[:], fb2[:, :F // 8], fb2[:, F // 8:])
                nc.vector.tensor_add(
                    remn[:, bi * RW:(bi + 1) * RW],
                    fb3[:, :F // 16], fb3[:, F // 16:])

            # ---------------- schedule ----------------
            # ACT: a few T bins fill time until t' is ready, then sigmoid(v),
            # then the rest.
            n_act_t_early = min(4, len(ACT_T))
            for j in range(n_act_t_early):
                emit_act_t(j)

            # DVE: early PE T-bins while z0/z1/y01 arrive (interleave solo)
            pe_seq = [("T", k) for k in PE_T] + [("R", k) for k in PE_R]
            solo_seq = list(enumerate(SOLO_T))
            pe_i = 0
            solo_i = 0
            n_early = 5
            for _ in range(min(n_early, len(pe_seq))):
                emit_pe_bin(*pe_seq[pe_i]); pe_i += 1

            # preprocessing: d = z1 - z0 ; t' = (y01 - 0.5) * d ; v = sigmoid(2 t')
            # (scalar_tensor_tensor runs 1x on DVE; tensor_scalar 4x +
            # tensor_tensor 2x pairs are faster)
            d = pool.tile([P, F], f16)
            nc.vector.tensor_sub(d[:], z1, z0)
            ysh = pool.tile([P, F], f16)
            nc.vector.tensor_scalar(
                out=ysh[:], in0=y01, scalar1=-0.5, scalar2=None, op0=Alu.add)
            tp = pool.tile([P, F], f16)
            nc.vector.tensor_mul(tp[:], ysh[:], d[:])
            v = pool.tile([P, F], f16)
            nc.scalar.activation(
                out=v[:], in_=tp[:], func=Act.Sigmoid, scale=2.0)
            # y = (g + 0.5) + v
            gsh = pool.tile([P, F], f16)
            nc.vector.tensor_scalar(
                out=gsh[:], in0=g[:], scalar1=0.5, scalar2=None, op0=Alu.add)
            y = pool.tile([P, F], f16)
            nc.vector.tensor_add(y[:], gsh[:], v[:])
            trash2 = pool.tile([P, F], f16)

            # rest of ACT bins
            for j in range(n_act_t_early, len(ACT_T)):
                emit_act_t(j)
            for j in range(len(ACT_R)):
                emit_act_r(j)

            # rest of DVE: interleave PE-bin gens with solo bins
            while pe_i < len(pe_seq) or solo_i < len(solo_seq):
                for _ in range(6):
                    if pe_i < len(pe_seq):
                        emit_pe_bin(*pe_seq[pe_i]); pe_i += 1
                if solo_i < len(solo_seq):
                    emit_solo_bin(*solo_seq[solo_i]); solo_i += 1

            # global cascade over solo remnants: [P, nsolo, w] fold to [P, nsolo]
            gb = remn
            w = RW
            while w > 1:
                half = w // 2
                src = gb[:].rearrange("p (g d) -> p g d", d=w)
                dt_lvl = f16 if half >= 2 else f32
                dst_t = pool.tile([P, max(nsolo, 1) * half], dt_lvl,
                                  name=f"gfold{w}")
                dst = dst_t[:].rearrange("p (g d) -> p g d", d=half)
                nc.vector.tensor_add(dst, src[:, :, :half], src[:, :, half:])
                gb = dst_t
                w = half

            # PSUM fold: copy to SBUF, tree 512 -> 1 (all PE bins at once)
            sb512 = pool.tile([P, 512], f32)
            nc.vector.tensor_copy(sb512[:], ps[:])
            cur = sb512
            wv = 512
            while wv > 1:
                nxt = pool.tile([P, wv // 2], f32, name=f"pf{wv//2}")
                nc.vector.tensor_add(nxt[:], cur[:, 0:wv // 2], cur[:, wv // 2:wv])
                cur = nxt
                wv //= 2

            # stage all outputs into one contiguous tile -> single DMA
            na = len(ACT_T) + len(ACT_R)
            stage = pool.tile([P, NOUT], f32)
            nc.vector.tensor_copy(stage[:, 0:1], cur[:])
            nc.vector.tensor_copy(stage[:, 1:1 + na], acc_act[:, 0:na])
            nc.vector.tensor_copy(stage[:, 1 + na:1 + na + nsolo], gb[:])
            nc.sync.dma_start(out=out_d[:], in_=stage[:])

    nc.compile()
    return nc


def _get_program():
    key = ("prog", tuple(SOLO_T), tuple(ACT_T), tuple(ACT_R))
    if key not in _prog_cache:
        _prog_cache[key] = _build_program()
    return _prog_cache[key]


def kernel(y_pred: np.ndarray, y: np.ndarray, voronoi: np.ndarray) -> np.ndarray:
    f16 = np.float16

    y_pred = np.asarray(y_pred, dtype=np.float32)
    y = np.asarray(y)
    voronoi = np.asarray(voronoi)

    nc = _get_program()

    nt = max(len(ACT_T), 1)
    nr = max(len(ACT_R), 1)
    bt = np.zeros(nt, np.float32)
    for j, k in enumerate(ACT_T):
        bt[j] = -30.0 * (k - 0.5)
    br = np.zeros(nr, np.float32)
    for j, k in enumerate(ACT_R):
        br[j] = -(k + 0.5)
    biast_np = np.broadcast_to(bt, (P, nt)).copy()
    biasr_np = np.broadcast_to(br, (P, nr)).copy()

    in_maps = []
    for c in range(NCORES):
        b = c // CORES_PER_SAMPLE
        q = c % CORES_PER_SAMPLE
        sl = slice(q * CHUNK, (q + 1) * CHUNK)
        zp = y_pred[b].reshape(C, N)
        zzy = np.empty((P, 3 * F), dtype=f16)
        zzy[:, 0:F] = zp[0, sl].astype(f16).reshape(P, F)
        zzy[:, F:2 * F] = zp[1, sl].astype(f16).reshape(P, F)
        zzy[:, 2 * F:3 * F] = y[b, 0].reshape(N)[sl].astype(f16).reshape(P, F)
        in_maps.append({
            "g": np.ascontiguousarray(
                voronoi[b].reshape(N)[sl]).astype(f16).reshape(P, F),
            "z0": zzy[:, 0:F].copy(),
            "z1": zzy[:, F:2 * F].copy(),
            "y01": zzy[:, 2 * F:3 * F].copy(),
            "biast": biast_np,
            "biasr": biasr_np,
        })

    res = bass_utils.run_bass_kernel_spmd(
        nc, in_maps, core_ids=list(range(NCORES)), trace=TRACE,
    )
    kernel.last_results = res

    # ---- host-side gather/unshard: combine per-core partials (f64) ----
    T = np.zeros((B, KMAX + 2), dtype=np.float64)
    R = np.zeros((B, KMAX + 2), dtype=np.float64)
    pe_seq = [("T", k) for k in PE_T] + [("R", k) for k in PE_R]
    na = len(ACT_T) + len(ACT_R)
    for c in range(NCORES):
        b = c // CORES_PER_SAMPLE
        out = np.asarray(res.results[c]["out"], dtype=np.float64)
        # PE bins: column 0, row r = bin scalar
        for r, (kind, k) in enumerate(pe_seq):
            val = out[r, 0]
            if kind == "T":
                T[b, k] += val
            else:
                R[b, k] += val
        # ACT bins: per-partition sums in cols 1..na
        acts = out[:, 1:1 + na].sum(axis=0)
        for j, k in enumerate(ACT_T):
            T[b, k] += acts[j]
        for j, k in enumerate(ACT_R):
            R[b, k] += acts[len(ACT_T) + j]
        # solo bins
        solo = out[:, 1 + na:1 + na + len(SOLO_T)].sum(axis=0)
        for j, k in enumerate(SOLO_T):
            T[b, k] += solo[j]

    scores = []
    ks = np.arange(1, KMAX + 1)
    for b in range(B):
        Tb = T[b].copy()
        Tb[KMAX + 1] = 0.0
        cnt = np.round(Tb[1:KMAX + 1] - Tb[2:KMAX + 2])  # cnt_k, k=1..64
        # V_k = R_k - sum_{j>=k} (j-k)*cnt_j  for k=1..65 (V_65 = 0)
        V = np.zeros(KMAX + 2)
        for k in range(1, KMAX + 1):
            j = np.arange(k, KMAX + 1)
            V[k] = R[b, k] - np.sum((j - k) * cnt[j - 1])
        inter = V[1:KMAX + 1] - V[2:KMAX + 2]
        dice = (2.0 * inter + EPS) / (2.0 * cnt + EPS)
        present = cnt > 0
        n_present = max(present.sum(), 1)
        scores.append(np.where(present, dice, 0.0).sum() / n_present)

    return np.float32(np.mean(scores))
